# revision 20
# baseline (speedup 1.0000x reference)
"""Trainium2 Bass kernel for a BERT block with low-rank (SVD) projections.

Strategy: pure batch-data-parallelism — 8 batch elements, one per NeuronCore,
no collectives. Device computation runs entirely in "transposed" activation
space [feature, token] so every matmul consumes weights in natural DRAM layout
as the stationary operand (lhsT) and no on-device transposes are needed; the
host transposes x / the output (cheap numpy) and pre-packs weights into
contiguous DMA tiles.

Engine budget: PE does all matmuls (f32r, free-dim 512); ACT only runs Exp /
Gelu (no activation-table thrash); DVE does PSUM->SBUF moves and LN math;
GPSIMD does the per-token broadcasts (softmax 1/denom, LN mu/rinv).
"""

import numpy as np

import concourse.bacc as bacc
import concourse.mybir as mybir
import concourse.tile as tile
from concourse.bass_utils import run_bass_kernel_spmd

F32 = mybir.dt.float32
F32R = mybir.dt.float32r
AF = mybir.ActivationFunctionType
OP = mybir.AluOpType

B, M, DM = 8, 512, 1024
H, DH = 16, 64
R_ATTN, R_WO, R_FF, DFF = 32, 512, 256, 4096
EPS = 1e-12
NFT = DM // 128      # 8 feature tiles
NMT = M // 128       # 4 token tiles
N_CORES = 8


# bias_all column layout (each column is one per-partition [128,1] vector)
BQP_COL = 0       # 8 cols: [bq_h0;bq_h1] per head pair
BKP_COL = 8       # 8 cols: [bk_h0;bk_h1] per head pair
BO_COL = 16       # 8 cols: bo_eff per feature tile
B1_COL = 24       # 32 cols: b1 per dff chunk
B2_COL = 56       # 8 cols: b2 per feature tile
LN1W_COL = 64
LN1B_COL = 72
LN2W_COL = 80
LN2B_COL = 88
BIAS_COLS = 96


def _emit(tc, nc, d, outT):
    ctx_pools = []

    def pool(name, bufs, space="SBUF"):
        p = tc.alloc_tile_pool(name=name, bufs=bufs, space=space)
        ctx_pools.append(p)
        return p

    const = pool("const", 1)
    bias_sb = const.tile([128, BIAS_COLS], F32, tag="bias")
    nc.sync.dma_start(out=bias_sb, in_=d["biasA"][:, :])
    mask_sb = const.tile([128, 4], F32, tag="mask")
    nc.sync.dma_start(out=mask_sb, in_=d["maskT"][:, :])
    ones_all = const.tile([128, M], F32R, tag="ones")
    nc.sync.dma_start(out=ones_all, in_=d["onesD"][:, :])
    ones_col = ones_all[:, 0:1]          # value 1/DM -> stats matmuls give means
    ones_row = ones_all[0:1, 1:129]      # value 1.0
    eps_tile = const.tile([1, 1], F32, tag="eps")
    nc.gpsimd.memset(eps_tile, EPS)

    # x^T resident: two [128, 2048] tiles, slice kt -> [:, (kt%4)*512 :...]
    xt_pool = pool("xt", 1)
    xt_tiles = []
    xT_r = d["xT"].rearrange("(k p) m -> k p m", p=128)
    for i in range(NFT):
        t = xt_pool.tile([128, M], F32R, tag=f"xt{i}", name=f"xt{i}")
        nc.sync.dma_start(out=t, in_=xT_r[i])
        xt_tiles.append(t)

    def xt(kt):
        return xt_tiles[kt]

    # attention output (transposed), resident
    attn_pool = pool("attn", 1)
    attn_sb = [attn_pool.tile([128, M], F32R, tag=f"attn{ft}", name=f"attn{ft}")
               for ft in range(NFT)]

    # ---------------- Attention ----------------
    wp = pool("wp", 8)          # big weight stage tiles [128, 1024]
    vb_pool = pool("vb", 1)
    # Vblk packed per proj: [128, 1024]; rows 0:64 = block-diag pair weights,
    # rows 64:128 = the same content (so base-64 operand pairs line up);
    # cols g*256 + j*128 + c
    vblk_sb = []
    for p in range(3):
        t = vb_pool.tile([128, 1024], F32R, tag=f"vb{p}", name=f"vb{p}")
        nc.sync.dma_start(out=t, in_=d["Vblk"][p])
        vblk_sb.append(t)

    ps_a = pool("ps_a", 2, space="PSUM")     # low / qk / v psums (shared tag)
    ps_s = pool("ps_s", 3, space="PSUM")     # scores
    ps_o = pool("ps_o", 2, space="PSUM")     # PV out + denom
    ps_rb = pool("ps_rb", 1, space="PSUM")   # 1/denom broadcast
    low_pool = pool("low", 12)
    qk_pool = pool("qk", 6)
    v_pool = pool("vsb", 8)
    p_pool = pool("pexp", 4)
    sm_pool = pool("sm", 4)

    # Phase A: all low-rank projections up front (dense PE burst, warms HAM).
    # low_sb[p][g]: [128, M] f32r; rows 64j:64j+64 = pair j's ranks.
    low_sb = [[None] * 4 for _ in range(3)]
    for g in range(4):
        for p in range(3):   # q, k, v
            wt = wp.tile([128, 1024], F32R, tag="pw")
            nc.sync.dma_start(out=wt, in_=d["Ppack"][p, g])
            ps_low = ps_a.tile([128, M], F32, tag="a")
            for kt in range(NFT):
                nc.tensor.matmul(
                    ps_low,
                    lhsT=wt[:, kt * 128:kt * 128 + 128],
                    rhs=xt(kt),
                    start=(kt == 0),
                    stop=(kt == NFT - 1),
                )
            t = low_pool.tile([128, M], F32R, tag="low")
            nc.vector.tensor_copy(t, ps_low)
            low_sb[p][g] = t

    # Phase B: per head-pair attention
    for g in range(4):
        for j in range(2):
            pr = 2 * g + j   # head pair index; heads (2*pr, 2*pr+1)
            cs = 256 * g + 128 * j
            # q, k pair tiles [128, M]; pair bias fused into the PSUM->SBUF move
            lo = slice(64 * j, 64 * j + 64)
            qk_sb = []
            for p in range(2):
                ps_qk = ps_a.tile([128, M], F32, tag="a")
                nc.tensor.matmul(
                    ps_qk,
                    lhsT=vblk_sb[p][lo, cs:cs + 128],
                    rhs=low_sb[p][g][lo, :],
                    start=True, stop=True,
                )
                col = (BQP_COL if p == 0 else BKP_COL) + pr
                t = qk_pool.tile([128, M], F32R, tag="qk")
                nc.vector.tensor_scalar_add(t, ps_qk, bias_sb[:, col:col + 1])
                qk_sb.append(t)
            q_sb, k_sb = qk_sb
            # v natural [tok, 2*(DH+1)] per token tile: [v_a|1|v_b|1]
            v_sb = []
            for mt in range(NMT):
                vt = v_pool.tile([128, 130], F32R, tag="vs")
                vt3 = vt.rearrange("p (h c) -> p h c", c=65)
                ps_v = ps_a.tile([128, 128], F32, tag="a")
                nc.tensor.matmul(
                    ps_v,
                    lhsT=low_sb[2][g][lo, mt * 128:mt * 128 + 128],
                    rhs=vblk_sb[2][lo, cs:cs + 128],
                    start=True, stop=True,
                )
                nc.vector.tensor_copy(
                    vt3[:, :, 0:64], ps_v.rearrange("p (h c) -> p h c", c=64))
                nc.vector.tensor_copy(
                    vt3[:, :, 64:65],
                    ones_all[:, 1:3].rearrange("p (h c) -> p h c", c=1))
                v_sb.append(vt)

            for a in range(2):
                h = 2 * pr + a
                po = ps_o.tile([65, M], F32, tag="o")
                for kt in range(NMT):
                    ps = ps_s.tile([128, M], F32, tag="s")
                    nc.tensor.matmul(
                        ps,
                        lhsT=k_sb[64 * a:64 * a + 64, kt * 128:kt * 128 + 128],
                        rhs=q_sb[64 * a:64 * a + 64, :],
                        start=True, stop=True,
                    )
                    pe = p_pool.tile([128, M], F32R, tag="pe")
                    nc.scalar.activation(
                        pe, ps, AF.Exp,
                        bias=mask_sb[:, kt:kt + 1], scale=1.0 / np.sqrt(DH),
                    )
                    nc.tensor.matmul(
                        po,
                        lhsT=v_sb[kt][:, 65 * a:65 * a + 65],
                        rhs=pe,
                        start=(kt == 0),
                        stop=(kt == NMT - 1),
                    )
                rd = sm_pool.tile([1, M], F32R, tag="rd")
                with nc.allow_low_precision(reason="f32r is 4-byte f32 storage"):
                    nc.vector.reciprocal(rd, po[64:65, :])
                prb = ps_rb.tile([64, M], F32, tag="rb")
                nc.tensor.matmul(prb, lhsT=ones_row[0:1, 0:64], rhs=rd,
                                 start=True, stop=True)
                rb_sb = sm_pool.tile([64, M], F32, tag="rbs")
                nc.vector.tensor_copy(rb_sb, prb)
                ft, rr = h // 2, h % 2
                nc.vector.tensor_tensor(
                    attn_sb[ft][64 * rr:64 * rr + 64, :],
                    po[0:64, :], rb_sb, op=OP.mult,
                )

    for p in (sm_pool, p_pool, v_pool, qk_pool, low_pool, ps_rb, ps_o, ps_s,
              ps_a, vb_pool):
        p.release()
        ctx_pools.remove(p)

    # ---------------- Output projection + LN1 ----------------
    ps_m = pool("ps_m", 2, space="PSUM")     # rotating [128, M] psums
    ps_st = pool("ps_st", 2, space="PSUM")   # LN stats [1, M]
    ps_w = pool("ps_w", 2, space="PSUM")     # FFN w^T accumulators
    ps_bc = pool("ps_bc", 2, space="PSUM")   # LN mu/rinv broadcasts

    wp2 = pool("wp2", 3)
    r_pool = pool("rp", 4)
    x1pre_pool = pool("x1pre", 1)
    x1_pool = pool("x1", 1)
    sq_pool = pool("sq", 2)
    t_pool = pool("tmp", 2)
    ln_pool = pool("ln", 4)

    # r^T = Uo^T @ attn^T   [RW, M]
    r_sb = []
    for mt in range(4):
        wt = wp.tile([128, 1024], F32R, tag="pw")
        nc.sync.dma_start(out=wt, in_=d["UoT"][mt])
        pr_ = ps_m.tile([128, M], F32, tag="m")
        for kt in range(NFT):
            nc.tensor.matmul(
                pr_, lhsT=wt[:, kt * 128:kt * 128 + 128], rhs=attn_sb[kt],
                start=(kt == 0), stop=(kt == NFT - 1),
            )
        t = r_pool.tile([128, M], F32R, tag="r")
        nc.vector.tensor_copy(t, pr_)
        r_sb.append(t)

    def layernorm(src_tiles, wcol, bcol, out_pool, out_tag, out_dt=F32R):
        """LN over the partition (feature) dim of 8 [128, M] tiles."""
        s1 = ps_st.tile([1, M], F32, tag="st")
        s2 = ps_st.tile([1, M], F32, tag="st")
        for ft in range(NFT):
            sq = sq_pool.tile([128, M], F32R, tag="sq")
            nc.vector.tensor_tensor(sq, src_tiles[ft], src_tiles[ft], op=OP.mult)
            nc.tensor.matmul(s1, lhsT=ones_col, rhs=src_tiles[ft],
                             start=(ft == 0), stop=(ft == NFT - 1))
            nc.tensor.matmul(s2, lhsT=ones_col, rhs=sq,
                             start=(ft == 0), stop=(ft == NFT - 1))
        # s1/s2 already hold means (ones_col carries 1/DM)
        mu_sb = ln_pool.tile([1, M], F32R, tag="mu")
        nc.vector.tensor_copy(mu_sb, s1)
        var = ln_pool.tile([1, M], F32, tag="var")
        nc.vector.tensor_tensor(var, mu_sb, mu_sb, op=OP.mult)
        nc.vector.tensor_tensor(var, s2, var, op=OP.subtract)
        sd = ln_pool.tile([1, M], F32, tag="sd")
        nc.scalar.activation(sd, var, AF.Sqrt, bias=eps_tile[0:1, 0:1])
        rinv = ln_pool.tile([1, M], F32R, tag="rinv")
        with nc.allow_low_precision(reason="f32r is 4-byte f32 storage"):
            nc.vector.reciprocal(rinv, sd)
        mu_bc = ps_bc.tile([128, M], F32, tag="bc")
        nc.tensor.matmul(mu_bc, lhsT=ones_row, rhs=mu_sb, start=True, stop=True)
        ri_bc = ps_bc.tile([128, M], F32, tag="bc")
        nc.tensor.matmul(ri_bc, lhsT=ones_row, rhs=rinv, start=True, stop=True)
        outs = []
        for ft in range(NFT):
            t = t_pool.tile([128, M], F32R, tag="t")
            nc.vector.tensor_tensor(t, src_tiles[ft], mu_bc, op=OP.subtract)
            nc.vector.tensor_tensor(t, t, ri_bc, op=OP.mult)
            o = out_pool.tile([128, M], out_dt, tag=f"{out_tag}{ft}")
            nc.vector.tensor_scalar(
                o, t,
                bias_sb[:, wcol + ft:wcol + ft + 1],
                bias_sb[:, bcol + ft:bcol + ft + 1],
                op0=OP.mult, op1=OP.add,
            )
            outs.append(o)
        return outs

    # x1pre^T = Vo^T @ r^T + bo_eff + x^T
    x1pre = []
    for ft in range(NFT):
        wt = wp2.tile([128, 512], F32R, tag="pw2")
        nc.sync.dma_start(out=wt, in_=d["VoT"][ft])
        px = ps_m.tile([128, M], F32, tag="m")
        for kt in range(4):
            nc.tensor.matmul(
                px, lhsT=wt[:, kt * 128:kt * 128 + 128], rhs=r_sb[kt],
                start=(kt == 0), stop=(kt == 3),
            )
        t = x1pre_pool.tile([128, M], F32R, tag=f"x1p{ft}")
        nc.vector.scalar_tensor_tensor(
            t, px, bias_sb[:, BO_COL + ft:BO_COL + ft + 1], xt(ft),
            op0=OP.add, op1=OP.add,
        )
        x1pre.append(t)

    x1 = layernorm(x1pre, LN1W_COL, LN1B_COL, x1_pool, "x1_")

    # ---------------- FFN ----------------
    u_pool = pool("up", 2)
    h_pool = pool("hp", 3)
    w_pool = pool("wsb", 2)
    z_pool = pool("zp", 1)
    out_pool = pool("outp", 1)

    u_sb = []
    for mt in range(2):
        wt = wp.tile([128, 1024], F32R, tag="pw")
        nc.sync.dma_start(out=wt, in_=d["U1T"][mt])
        pu = ps_m.tile([128, M], F32, tag="m")
        for kt in range(NFT):
            nc.tensor.matmul(
                pu, lhsT=wt[:, kt * 128:kt * 128 + 128], rhs=x1[kt],
                start=(kt == 0), stop=(kt == NFT - 1),
            )
        t = u_pool.tile([128, M], F32R, tag="u")
        nc.vector.tensor_copy(t, pu)
        u_sb.append(t)

    pw0 = ps_w.tile([128, M], F32, tag="w")
    pw1 = ps_w.tile([128, M], F32, tag="w")
    for cg in range(4):       # chunk groups of 8 dff-chunks
        v1t = [None, None]
        for kt in range(2):
            v1t[kt] = wp.tile([128, 1024], F32R, tag="pw", name="v1t")
            nc.sync.dma_start(out=v1t[kt], in_=d["V1T"][kt, cg])
        u2t = [None, None]
        for mt in range(2):
            u2t[mt] = wp.tile([128, 1024], F32R, tag="pw", name="u2t")
            nc.sync.dma_start(out=u2t[mt], in_=d["U2T"][cg, mt])
        for c8 in range(8):
            ct = cg * 8 + c8
            ph = ps_m.tile([128, M], F32, tag="m")
            for kt in range(2):
                nc.tensor.matmul(
                    ph, lhsT=v1t[kt][:, c8 * 128:c8 * 128 + 128], rhs=u_sb[kt],
                    start=(kt == 0), stop=(kt == 1),
                )
            ht = h_pool.tile([128, M], F32R, tag="h")
            nc.scalar.activation(
                ht, ph, AF.Gelu, bias=bias_sb[:, B1_COL + ct:B1_COL + ct + 1],
            )
            for mt, pw_ in enumerate((pw0, pw1)):
                nc.tensor.matmul(
                    pw_, lhsT=u2t[mt][:, c8 * 128:c8 * 128 + 128], rhs=ht,
                    start=(ct == 0), stop=(ct == 31),
                )
    w_sb = []
    for mt, pw_ in enumerate((pw0, pw1)):
        t = w_pool.tile([128, M], F32R, tag="w")
        nc.vector.tensor_copy(t, pw_)
        w_sb.append(t)

    # y^T = V2^T @ w^T + b2 + x1  -> z
    z = []
    v2t = [None, None]
    for kt in range(2):
        v2t[kt] = wp.tile([128, 1024], F32R, tag="pw", name="v2t")
        nc.sync.dma_start(out=v2t[kt], in_=d["V2T"][kt])
    for ft in range(NFT):
        py = ps_m.tile([128, M], F32, tag="m")
        for kt in range(2):
            nc.tensor.matmul(
                py, lhsT=v2t[kt][:, ft * 128:ft * 128 + 128], rhs=w_sb[kt],
                start=(kt == 0), stop=(kt == 1),
            )
        t = z_pool.tile([128, M], F32R, tag=f"z{ft}")
        nc.vector.scalar_tensor_tensor(
            t, py, bias_sb[:, B2_COL + ft:B2_COL + ft + 1], x1[ft],
            op0=OP.add, op1=OP.add,
        )
        z.append(t)

    out_tiles = layernorm(z, LN2W_COL, LN2B_COL, out_pool, "o_", out_dt=F32)
    for ft in range(NFT):
        nc.sync.dma_start(out=outT[ft * 128:ft * 128 + 128, :], in_=out_tiles[ft])

    for p in reversed(ctx_pools):
        p.release()


def build_program():
    nc = bacc.Bacc("TRN2", target_bir_lowering=False, debug=False)
    d = {}

    def din(name, shape, dt=F32R):
        d[name] = nc.dram_tensor(name, list(shape), dt, kind="ExternalInput")
        return d[name]

    din("xT", (DM, M))
    din("maskT", (128, 4), F32)
    din("onesD", (128, M))
    din("biasA", (128, BIAS_COLS), F32)
    din("Ppack", (3, 4, 128, 1024))
    din("Vblk", (3, 128, 1024))
    din("UoT", (4, 128, 1024))
    din("VoT", (8, 128, 512))
    din("U1T", (2, 128, 1024))
    din("V1T", (2, 4, 128, 1024))
    din("U2T", (4, 2, 128, 1024))
    din("V2T", (2, 128, 1024))
    outT = nc.dram_tensor("outT", [DM, M], F32, kind="ExternalOutput")
    with tile.TileContext(nc) as tc:
        _emit(tc, nc, d, outT)
    nc.compile()
    return nc


def host_pack_weights(inp):
    """Pack all weights into contiguous DMA-friendly arrays (shared by cores)."""
    f = np.float32
    W = {}
    # Ppack [3,4,128,1024]: (proj, group, dm_partition, kt*128 + 4heads*32)
    pp = np.empty((3, 4, 128, 1024), f)
    for p, name in enumerate(("Pq", "Pk", "Pv")):
        P = np.asarray(inp[name], f)          # [16, 1024, 32]
        for g in range(4):
            grp = np.concatenate([P[4 * g + i] for i in range(4)], axis=1)  # [1024,128]
            pp[p, g] = grp.reshape(8, 128, 128).transpose(1, 0, 2).reshape(128, 1024)
    W["Ppack"] = pp
    # Vblk [3, 128, 1024]: rows 0:64 block-diag pairs, rows 64:128 duplicate;
    # cols g*256 + j*128 + c
    vb = np.zeros((3, 128, 1024), f)
    for p, name in enumerate(("Vq", "Vk", "Vv")):
        V = np.asarray(inp[name], f)          # [16, 32, 64]
        for g in range(4):
            for j in range(2):
                h0, h1 = 4 * g + 2 * j, 4 * g + 2 * j + 1
                c0 = 256 * g + 128 * j
                vb[p, 0:32, c0:c0 + 64] = V[h0]
                vb[p, 32:64, c0 + 64:c0 + 128] = V[h1]
    vb[:, 64:128, :] = vb[:, 0:64, :]
    W["Vblk"] = vb
    Uo = np.asarray(inp["Uo"], f)
    Vo = np.asarray(inp["Vo"], f)
    W["UoT"] = Uo.reshape(8, 128, 4, 128).transpose(2, 1, 0, 3).reshape(4, 128, 1024)
    W["VoT"] = Vo.reshape(4, 128, 8, 128).transpose(2, 1, 0, 3).reshape(8, 128, 512)
    U1 = np.asarray(inp["U1"], f)
    W["U1T"] = U1.reshape(8, 128, 2, 128).transpose(2, 1, 0, 3).reshape(2, 128, 1024)
    V1 = np.asarray(inp["V1"], f)
    W["V1T"] = V1.reshape(2, 128, 4, 8, 128).transpose(0, 2, 1, 3, 4).reshape(2, 4, 128, 1024)
    U2 = np.asarray(inp["U2"], f)
    W["U2T"] = U2.reshape(4, 8, 128, 2, 128).transpose(0, 3, 2, 1, 4).reshape(4, 2, 128, 1024)
    V2 = np.asarray(inp["V2"], f)
    W["V2T"] = np.ascontiguousarray(V2.reshape(2, 128, 1024))

    # bias_all [128, 96]
    ba = np.zeros((128, BIAS_COLS), f)
    bq = np.asarray(inp["bq"], f)
    bk = np.asarray(inp["bk"], f)
    for r_ in range(8):
        ba[:, BQP_COL + r_] = np.concatenate([bq[2 * r_], bq[2 * r_ + 1]])
        ba[:, BKP_COL + r_] = np.concatenate([bk[2 * r_], bk[2 * r_ + 1]])
    bv_full = np.asarray(inp["bv"], f).reshape(-1)
    bo_eff = np.asarray(inp["bo_attn"], f) + bv_full @ Uo @ Vo
    ba[:, BO_COL:BO_COL + 8] = bo_eff.reshape(8, 128).T
    ba[:, B1_COL:B1_COL + 32] = np.asarray(inp["b1"], f).reshape(32, 128).T
    ba[:, B2_COL:B2_COL + 8] = np.asarray(inp["b2"], f).reshape(8, 128).T
    ba[:, LN1W_COL:LN1W_COL + 8] = np.asarray(inp["ln1_w"], f).reshape(8, 128).T
    ba[:, LN1B_COL:LN1B_COL + 8] = np.asarray(inp["ln1_b"], f).reshape(8, 128).T
    ba[:, LN2W_COL:LN2W_COL + 8] = np.asarray(inp["ln2_w"], f).reshape(8, 128).T
    ba[:, LN2B_COL:LN2B_COL + 8] = np.asarray(inp["ln2_b"], f).reshape(8, 128).T
    W["biasA"] = ba
    ones = np.ones((128, M), np.float32)
    ones[:, 0] = 1.0 / DM            # ones_col used by LN stats -> means
    W["onesD"] = ones
    return W


def make_in_maps(inputs):
    W = host_pack_weights(inputs)
    x = np.asarray(inputs["x"], np.float32)
    mask = np.asarray(inputs["mask"], np.float32)
    in_maps = []
    for b in range(N_CORES):
        m = dict(W)
        m["xT"] = np.ascontiguousarray(x[b].T)
        m["maskT"] = np.ascontiguousarray(mask[b].reshape(4, 128).T)
        in_maps.append(m)
    return in_maps


_NC = None


def _get_nc():
    global _NC
    if _NC is None:
        _NC = build_program()
    return _NC


def run(inputs, trace=False):
    nc = _get_nc()
    in_maps = make_in_maps(inputs)
    bkr = run_bass_kernel_spmd(nc, in_maps, list(range(N_CORES)), trace=trace)
    out = np.empty((B, M, DM), np.float32)
    for b in range(N_CORES):
        out[b] = bkr.results[b]["outT"].T
    return out, bkr


def kernel(**inputs):
    out, _ = run(inputs)
    return out


# revision 28
# speedup vs baseline: 1.1028x; 1.1028x over previous
"""Trainium2 Bass kernel for a BERT block with low-rank (SVD) projections.

Strategy: pure batch-data-parallelism — 8 batch elements, one per NeuronCore,
no collectives. Device computation runs entirely in "transposed" activation
space [feature, token] so every matmul consumes weights in natural DRAM layout
as the stationary operand (lhsT) and no on-device transposes are needed; the
host transposes x / the output (cheap numpy) and pre-packs weights into
contiguous DMA tiles.

Engine budget: PE does all matmuls (f32r, free-dim 512); ACT only runs Exp /
Gelu (no activation-table thrash); DVE does PSUM->SBUF moves and LN math;
GPSIMD does the per-token broadcasts (softmax 1/denom, LN mu/rinv).
"""

import numpy as np

import concourse.bacc as bacc
import concourse.mybir as mybir
import concourse.tile as tile
from concourse.bass_utils import run_bass_kernel_spmd

F32 = mybir.dt.float32
F32R = mybir.dt.float32r
AF = mybir.ActivationFunctionType
OP = mybir.AluOpType

B, M, DM = 8, 512, 1024
H, DH = 16, 64
R_ATTN, R_WO, R_FF, DFF = 32, 512, 256, 4096
EPS = 1e-12
NFT = DM // 128      # 8 feature tiles
NMT = M // 128       # 4 token tiles
N_CORES = 8


# bias_all column layout (each column is one per-partition [128,1] vector)
BQP_COL = 0       # 8 cols: [bq_h0;bq_h1] per head pair
BKP_COL = 8       # 8 cols: [bk_h0;bk_h1] per head pair
BO_COL = 16       # 8 cols: bo_eff per feature tile
B1_COL = 24       # 32 cols: b1 per dff chunk
B2_COL = 56       # 8 cols: b2 per feature tile
LN1W_COL = 64
LN1B_COL = 72
LN2W_COL = 80
LN2B_COL = 88
BIAS_COLS = 96


def _emit(tc, nc, d, outT):
    ctx_pools = []

    def pool(name, bufs, space="SBUF"):
        p = tc.alloc_tile_pool(name=name, bufs=bufs, space=space)
        ctx_pools.append(p)
        return p

    const = pool("const", 1)
    bias_sb = const.tile([128, BIAS_COLS], F32, tag="bias")
    nc.sync.dma_start(out=bias_sb, in_=d["biasA"][:, :])
    mask_sb = const.tile([128, 4], F32, tag="mask")
    nc.sync.dma_start(out=mask_sb, in_=d["maskT"][:, :])
    ones_all = const.tile([128, M], F32R, tag="ones")
    nc.sync.dma_start(out=ones_all, in_=d["onesD"][:, :])
    ones_col = ones_all[:, 0:1]          # value 1/DM -> stats matmuls give means
    ones_row = ones_all[0:1, 1:129]      # value 1.0
    eps_tile = const.tile([1, 1], F32, tag="eps")
    nc.gpsimd.memset(eps_tile, EPS)
    zero_col = ones_all[:, 3:4]          # value 0.0 (bias operand for ACT Square)
    # selD2 [128, 384] f32: cols 0:128 pattern A, 128:256 pattern B (softmax
    # denom broadcast selectors), row 0 of cols 256:384 = ones (LN broadcast)
    sel_sb = const.tile([128, 384], F32, tag="sel")
    nc.sync.dma_start(out=sel_sb, in_=d["selD"][:, :])
    ones_row_f = sel_sb[0:1, 256:384]

    # x^T resident: two [128, 2048] tiles, slice kt -> [:, (kt%4)*512 :...]
    xt_pool = pool("xt", 1)
    xt_tiles = []
    xT_r = d["xT"].rearrange("(k p) m -> k p m", p=128)
    for i in range(NFT):
        t = xt_pool.tile([128, M], F32R, tag=f"xt{i}", name=f"xt{i}")
        nc.sync.dma_start(out=t, in_=xT_r[i])
        xt_tiles.append(t)

    def xt(kt):
        return xt_tiles[kt]

    # attention output (transposed), resident
    attn_pool = pool("attn", 1)
    attn_sb = [attn_pool.tile([128, M], F32R, tag=f"attn{ft}", name=f"attn{ft}")
               for ft in range(NFT)]

    # ---------------- Attention ----------------
    wp = pool("wp", 8)          # big weight stage tiles [128, 1024]
    vb_pool = pool("vb", 1)
    # Vblk packed per proj: [128, 1024]; rows 0:64 = block-diag pair weights,
    # rows 64:128 = the same content (so base-64 operand pairs line up);
    # cols g*256 + j*128 + c
    vblk_sb = []
    for p in range(3):
        t = vb_pool.tile([128, 1024], F32R, tag=f"vb{p}", name=f"vb{p}")
        nc.sync.dma_start(out=t, in_=d["Vblk"][p])
        vblk_sb.append(t)

    ps_a = pool("ps_a", 2, space="PSUM")     # low / qk / v psums (shared tag)
    ps_s = pool("ps_s", 3, space="PSUM")     # scores
    ps_o = pool("ps_o", 2, space="PSUM")     # PV out + denom
    ps_rb = pool("ps_rb", 1, space="PSUM")   # 1/denom broadcast
    low_pool = pool("low", 12)
    qk_pool = pool("qk", 6)
    v_pool = pool("vsb", 8)
    p_pool = pool("pexp", 4)
    sm_pool = pool("sm", 4)
    # softmax denom collection: head h -> den4[h//4] row (h%4)*32
    rec_pool = pool("rec", 1)
    den4, rec4 = [], []
    for i in range(4):
        t = rec_pool.tile([128, M], F32, tag=f"den{i}", name=f"den{i}")
        nc.gpsimd.memset(t, 1.0)
        den4.append(t)
        t2 = rec_pool.tile([128, M], F32, tag=f"rec{i}", name=f"rec{i}")
        rec4.append(t2)

    # Phase A: all low-rank projections up front (dense PE burst, warms HAM).
    # low_sb[p][g]: [128, M] f32r; rows 64j:64j+64 = pair j's ranks.
    low_sb = [[None] * 4 for _ in range(3)]
    for g in range(4):
        for p in range(3):   # q, k, v
            wt = wp.tile([128, 1024], F32R, tag="pw")
            nc.sync.dma_start(out=wt, in_=d["Ppack"][p, g])
            ps_low = ps_a.tile([128, M], F32, tag="a")
            for kt in range(NFT):
                nc.tensor.matmul(
                    ps_low,
                    lhsT=wt[:, kt * 128:kt * 128 + 128],
                    rhs=xt(kt),
                    start=(kt == 0),
                    stop=(kt == NFT - 1),
                )
            t = low_pool.tile([128, M], F32R, tag="low")
            nc.vector.tensor_copy(t, ps_low)
            low_sb[p][g] = t

    # Phase B: per head-pair attention
    for g in range(4):
        for j in range(2):
            pr = 2 * g + j   # head pair index; heads (2*pr, 2*pr+1)
            cs = 256 * g + 128 * j
            # q, k pair tiles [128, M]; pair bias fused into the PSUM->SBUF move
            lo = slice(64 * j, 64 * j + 64)
            qk_sb = []
            for p in range(2):
                ps_qk = ps_a.tile([128, M], F32, tag="a")
                nc.tensor.matmul(
                    ps_qk,
                    lhsT=vblk_sb[p][lo, cs:cs + 128],
                    rhs=low_sb[p][g][lo, :],
                    start=True, stop=True,
                )
                col = (BQP_COL if p == 0 else BKP_COL) + pr
                t = qk_pool.tile([128, M], F32R, tag="qk")
                nc.vector.tensor_scalar_add(t, ps_qk, bias_sb[:, col:col + 1])
                qk_sb.append(t)
            q_sb, k_sb = qk_sb
            # v natural [tok, 2*(DH+1)] per token tile: [v_a|1|v_b|1]
            v_sb = []
            for mt in range(NMT):
                vt = v_pool.tile([128, 130], F32R, tag="vs")
                vt3 = vt.rearrange("p (h c) -> p h c", c=65)
                ps_v = ps_a.tile([128, 128], F32, tag="a")
                nc.tensor.matmul(
                    ps_v,
                    lhsT=low_sb[2][g][lo, mt * 128:mt * 128 + 128],
                    rhs=vblk_sb[2][lo, cs:cs + 128],
                    start=True, stop=True,
                )
                nc.vector.tensor_copy(
                    vt3[:, :, 0:64], ps_v.rearrange("p (h c) -> p h c", c=64))
                nc.vector.tensor_copy(
                    vt3[:, :, 64:65],
                    ones_all[:, 1:3].rearrange("p (h c) -> p h c", c=1))
                v_sb.append(vt)

            for a in range(2):
                h = 2 * pr + a
                po = ps_o.tile([65, M], F32, tag="o")
                for kt in range(NMT):
                    ps = ps_s.tile([128, M], F32, tag="s")
                    nc.tensor.matmul(
                        ps,
                        lhsT=k_sb[64 * a:64 * a + 64, kt * 128:kt * 128 + 128],
                        rhs=q_sb[64 * a:64 * a + 64, :],
                        start=True, stop=True,
                    )
                    pe = p_pool.tile([128, M], F32R, tag="pe")
                    nc.scalar.activation(
                        pe, ps, AF.Exp,
                        bias=mask_sb[:, kt:kt + 1], scale=1.0 / np.sqrt(DH),
                    )
                    nc.tensor.matmul(
                        po,
                        lhsT=v_sb[kt][:, 65 * a:65 * a + 65],
                        rhs=pe,
                        start=(kt == 0),
                        stop=(kt == NMT - 1),
                    )
                ft, rr = h // 2, h % 2
                ro = (h % 4) * 32
                nc.vector.tensor_copy(den4[h // 4][ro:ro + 1, :], po[64:65, :])
                nc.vector.tensor_copy(
                    attn_sb[ft][64 * rr:64 * rr + 64, :], po[0:64, :])

    # batched softmax normalization: 4 full-tile fast reciprocals, then
    # per-feature-tile PE broadcast + one in-place multiply
    for i in range(4):
        nc.vector.reciprocal_approx_fast(out=rec4[i], in_=den4[i])
    for ft in range(NFT):
        pat = ft % 2
        prb = ps_rb.tile([128, M], F32, tag="rb")
        nc.tensor.matmul(prb, lhsT=sel_sb[:, 128 * pat:128 * pat + 128],
                         rhs=rec4[ft // 2], start=True, stop=True)
        nc.vector.tensor_tensor(attn_sb[ft], attn_sb[ft], prb, op=OP.mult)

    for p in (rec_pool, sm_pool, p_pool, v_pool, qk_pool, low_pool, ps_rb,
              ps_o, ps_s, ps_a, vb_pool):
        p.release()
        ctx_pools.remove(p)

    # ---------------- Output projection + LN1 ----------------
    ps_m = pool("ps_m", 2, space="PSUM")     # rotating [128, M] psums
    ps_st = pool("ps_st", 2, space="PSUM")   # LN stats [1, M]
    ps_w = pool("ps_w", 2, space="PSUM")     # FFN w^T accumulators
    ps_bc = pool("ps_bc", 2, space="PSUM")   # LN mu/rinv broadcasts

    wp2 = pool("wp2", 3)
    x1_pool = pool("x1", 1)
    sq_pool = pool("sq", 2)
    t_pool = pool("tmp", 2)
    ln_pool = pool("ln", 4)
    x1pre_pool = pool("x1pre", 1)
    r_pool = pool("rp", 4)

    # r^T = Uo^T @ attn^T   [RW, M]
    r_sb = []
    for mt in range(4):
        wt = wp.tile([128, 1024], F32R, tag="pw")
        nc.sync.dma_start(out=wt, in_=d["UoT"][mt])
        pr_ = ps_m.tile([128, M], F32, tag="m")
        for kt in range(NFT):
            nc.tensor.matmul(
                pr_, lhsT=wt[:, kt * 128:kt * 128 + 128], rhs=attn_sb[kt],
                start=(kt == 0), stop=(kt == NFT - 1),
            )
        t = r_pool.tile([128, M], F32R, tag="r")
        nc.vector.tensor_copy(t, pr_)
        r_sb.append(t)

    def layernorm(src_tiles, wcol, bcol, out_pool, out_tag, out_dt=F32R):
        """LN over the partition (feature) dim of 8 [128, M] tiles."""
        s1 = ps_st.tile([1, M], F32, tag="st")
        s2 = ps_st.tile([1, M], F32, tag="st")
        for ft in range(NFT):
            sq = sq_pool.tile([128, M], F32R, tag="sq")
            nc.scalar.activation(sq, src_tiles[ft], AF.Square, bias=zero_col)
            nc.tensor.matmul(s1, lhsT=ones_col, rhs=src_tiles[ft],
                             start=(ft == 0), stop=(ft == NFT - 1))
            nc.tensor.matmul(s2, lhsT=ones_col, rhs=sq,
                             start=(ft == 0), stop=(ft == NFT - 1))
        # s1/s2 already hold means (ones_col carries 1/DM)
        mu_sb = ln_pool.tile([1, M], F32, tag="mu")
        nc.vector.tensor_copy(mu_sb, s1)
        var = ln_pool.tile([1, M], F32, tag="var")
        nc.vector.tensor_tensor(var, mu_sb, mu_sb, op=OP.mult)
        nc.vector.tensor_tensor(var, s2, var, op=OP.subtract)
        sd = ln_pool.tile([1, M], F32, tag="sd")
        nc.scalar.activation(sd, var, AF.Sqrt, bias=eps_tile[0:1, 0:1])
        rinv_f = ln_pool.tile([1, M], F32, tag="rinvf")
        nc.vector.reciprocal_approx_fast(out=rinv_f, in_=sd)
        mu_bc = ps_bc.tile([128, M], F32, tag="bc")
        nc.tensor.matmul(mu_bc, lhsT=ones_row_f, rhs=mu_sb, start=True, stop=True)
        ri_bc = ps_bc.tile([128, M], F32, tag="bc")
        nc.tensor.matmul(ri_bc, lhsT=ones_row_f, rhs=rinv_f, start=True, stop=True)
        outs = []
        for ft in range(NFT):
            t = t_pool.tile([128, M], F32R, tag="t")
            nc.vector.tensor_tensor(t, src_tiles[ft], mu_bc, op=OP.subtract)
            nc.vector.tensor_tensor(t, t, ri_bc, op=OP.mult)
            o = out_pool.tile([128, M], out_dt, tag=f"{out_tag}{ft}")
            nc.scalar.activation(
                o, t, AF.Identity,
                bias=bias_sb[:, bcol + ft:bcol + ft + 1],
                scale=bias_sb[:, wcol + ft:wcol + ft + 1],
            )
            outs.append(o)
        return outs

    # x1pre^T = Vo^T @ r^T + bo_eff + x^T
    x1pre = []
    for ft in range(NFT):
        wt = wp2.tile([128, 512], F32R, tag="pw2")
        nc.sync.dma_start(out=wt, in_=d["VoT"][ft])
        px = ps_m.tile([128, M], F32, tag="m")
        for kt in range(4):
            nc.tensor.matmul(
                px, lhsT=wt[:, kt * 128:kt * 128 + 128], rhs=r_sb[kt],
                start=(kt == 0), stop=(kt == 3),
            )
        t = x1pre_pool.tile([128, M], F32R, tag=f"x1p{ft}")
        nc.vector.scalar_tensor_tensor(
            t, px, bias_sb[:, BO_COL + ft:BO_COL + ft + 1], xt(ft),
            op0=OP.add, op1=OP.add,
        )
        x1pre.append(t)

    r_pool.release()
    ctx_pools.remove(r_pool)
    x1 = layernorm(x1pre, LN1W_COL, LN1B_COL, x1_pool, "x1_")
    x1pre_pool.release()
    ctx_pools.remove(x1pre_pool)

    # ---------------- FFN ----------------
    u_pool = pool("up", 2)
    h_pool = pool("hp", 3)
    w_pool = pool("wsb", 2)
    z_pool = pool("zp", 1)
    out_pool = pool("outp", 1)

    u_sb = []
    for mt in range(2):
        wt = wp.tile([128, 1024], F32R, tag="pw")
        nc.sync.dma_start(out=wt, in_=d["U1T"][mt])
        pu = ps_m.tile([128, M], F32, tag="m")
        for kt in range(NFT):
            nc.tensor.matmul(
                pu, lhsT=wt[:, kt * 128:kt * 128 + 128], rhs=x1[kt],
                start=(kt == 0), stop=(kt == NFT - 1),
            )
        t = u_pool.tile([128, M], F32R, tag="u")
        nc.vector.tensor_copy(t, pu)
        u_sb.append(t)

    pw0 = ps_w.tile([128, M], F32, tag="w")
    pw1 = ps_w.tile([128, M], F32, tag="w")
    for cg in range(4):       # chunk groups of 8 dff-chunks
        v1t = [None, None]
        for kt in range(2):
            v1t[kt] = wp.tile([128, 1024], F32R, tag="pw", name="v1t")
            nc.sync.dma_start(out=v1t[kt], in_=d["V1T"][kt, cg])
        u2t = [None, None]
        for mt in range(2):
            u2t[mt] = wp.tile([128, 1024], F32R, tag="pw", name="u2t")
            nc.sync.dma_start(out=u2t[mt], in_=d["U2T"][cg, mt])
        for c8 in range(8):
            ct = cg * 8 + c8
            ph = ps_m.tile([128, M], F32, tag="m")
            for kt in range(2):
                nc.tensor.matmul(
                    ph, lhsT=v1t[kt][:, c8 * 128:c8 * 128 + 128], rhs=u_sb[kt],
                    start=(kt == 0), stop=(kt == 1),
                )
            ht = h_pool.tile([128, M], F32R, tag="h")
            nc.scalar.activation(
                ht, ph, AF.Gelu, bias=bias_sb[:, B1_COL + ct:B1_COL + ct + 1],
            )
            for mt, pw_ in enumerate((pw0, pw1)):
                nc.tensor.matmul(
                    pw_, lhsT=u2t[mt][:, c8 * 128:c8 * 128 + 128], rhs=ht,
                    start=(ct == 0), stop=(ct == 31),
                )
    w_sb = []
    for mt, pw_ in enumerate((pw0, pw1)):
        t = w_pool.tile([128, M], F32R, tag="w")
        nc.vector.tensor_copy(t, pw_)
        w_sb.append(t)

    # y^T = V2^T @ w^T + b2 + x1  -> z
    z = []
    v2t = [None, None]
    for kt in range(2):
        v2t[kt] = wp.tile([128, 1024], F32R, tag="pw", name="v2t")
        nc.sync.dma_start(out=v2t[kt], in_=d["V2T"][kt])
    for ft in range(NFT):
        py = ps_m.tile([128, M], F32, tag="m")
        for kt in range(2):
            nc.tensor.matmul(
                py, lhsT=v2t[kt][:, ft * 128:ft * 128 + 128], rhs=w_sb[kt],
                start=(kt == 0), stop=(kt == 1),
            )
        t = z_pool.tile([128, M], F32R, tag=f"z{ft}")
        nc.vector.scalar_tensor_tensor(
            t, py, bias_sb[:, B2_COL + ft:B2_COL + ft + 1], x1[ft],
            op0=OP.add, op1=OP.add,
        )
        z.append(t)

    out_tiles = layernorm(z, LN2W_COL, LN2B_COL, out_pool, "o_", out_dt=F32)
    for ft in range(NFT):
        nc.sync.dma_start(out=outT[ft * 128:ft * 128 + 128, :], in_=out_tiles[ft])

    for p in reversed(ctx_pools):
        p.release()


def build_program():
    nc = bacc.Bacc("TRN2", target_bir_lowering=False, debug=False)
    d = {}

    def din(name, shape, dt=F32R):
        d[name] = nc.dram_tensor(name, list(shape), dt, kind="ExternalInput")
        return d[name]

    din("xT", (DM, M))
    din("maskT", (128, 4), F32)
    din("onesD", (128, M))
    din("selD", (128, 384), F32)
    din("biasA", (128, BIAS_COLS), F32)
    din("Ppack", (3, 4, 128, 1024))
    din("Vblk", (3, 128, 1024))
    din("UoT", (4, 128, 1024))
    din("VoT", (8, 128, 512))
    din("U1T", (2, 128, 1024))
    din("V1T", (2, 4, 128, 1024))
    din("U2T", (4, 2, 128, 1024))
    din("V2T", (2, 128, 1024))
    outT = nc.dram_tensor("outT", [DM, M], F32, kind="ExternalOutput")
    with tile.TileContext(nc) as tc:
        _emit(tc, nc, d, outT)
    nc.compile()
    return nc


def host_pack_weights(inp):
    """Pack all weights into contiguous DMA-friendly arrays (shared by cores)."""
    f = np.float32
    W = {}
    # Ppack [3,4,128,1024]: (proj, group, dm_partition, kt*128 + 4heads*32)
    pp = np.empty((3, 4, 128, 1024), f)
    for p, name in enumerate(("Pq", "Pk", "Pv")):
        P = np.asarray(inp[name], f)          # [16, 1024, 32]
        for g in range(4):
            grp = np.concatenate([P[4 * g + i] for i in range(4)], axis=1)  # [1024,128]
            pp[p, g] = grp.reshape(8, 128, 128).transpose(1, 0, 2).reshape(128, 1024)
    W["Ppack"] = pp
    # Vblk [3, 128, 1024]: rows 0:64 block-diag pairs, rows 64:128 duplicate;
    # cols g*256 + j*128 + c
    vb = np.zeros((3, 128, 1024), f)
    for p, name in enumerate(("Vq", "Vk", "Vv")):
        V = np.asarray(inp[name], f)          # [16, 32, 64]
        for g in range(4):
            for j in range(2):
                h0, h1 = 4 * g + 2 * j, 4 * g + 2 * j + 1
                c0 = 256 * g + 128 * j
                vb[p, 0:32, c0:c0 + 64] = V[h0]
                vb[p, 32:64, c0 + 64:c0 + 128] = V[h1]
    vb[:, 64:128, :] = vb[:, 0:64, :]
    W["Vblk"] = vb
    Uo = np.asarray(inp["Uo"], f)
    Vo = np.asarray(inp["Vo"], f)
    W["UoT"] = Uo.reshape(8, 128, 4, 128).transpose(2, 1, 0, 3).reshape(4, 128, 1024)
    W["VoT"] = Vo.reshape(4, 128, 8, 128).transpose(2, 1, 0, 3).reshape(8, 128, 512)
    U1 = np.asarray(inp["U1"], f)
    W["U1T"] = U1.reshape(8, 128, 2, 128).transpose(2, 1, 0, 3).reshape(2, 128, 1024)
    V1 = np.asarray(inp["V1"], f)
    W["V1T"] = V1.reshape(2, 128, 4, 8, 128).transpose(0, 2, 1, 3, 4).reshape(2, 4, 128, 1024)
    U2 = np.asarray(inp["U2"], f)
    W["U2T"] = U2.reshape(4, 8, 128, 2, 128).transpose(0, 3, 2, 1, 4).reshape(4, 2, 128, 1024)
    V2 = np.asarray(inp["V2"], f)
    W["V2T"] = np.ascontiguousarray(V2.reshape(2, 128, 1024))

    # bias_all [128, 96]
    ba = np.zeros((128, BIAS_COLS), f)
    bq = np.asarray(inp["bq"], f)
    bk = np.asarray(inp["bk"], f)
    for r_ in range(8):
        ba[:, BQP_COL + r_] = np.concatenate([bq[2 * r_], bq[2 * r_ + 1]])
        ba[:, BKP_COL + r_] = np.concatenate([bk[2 * r_], bk[2 * r_ + 1]])
    bv_full = np.asarray(inp["bv"], f).reshape(-1)
    bo_eff = np.asarray(inp["bo_attn"], f) + bv_full @ Uo @ Vo
    ba[:, BO_COL:BO_COL + 8] = bo_eff.reshape(8, 128).T
    ba[:, B1_COL:B1_COL + 32] = np.asarray(inp["b1"], f).reshape(32, 128).T
    ba[:, B2_COL:B2_COL + 8] = np.asarray(inp["b2"], f).reshape(8, 128).T
    ba[:, LN1W_COL:LN1W_COL + 8] = np.asarray(inp["ln1_w"], f).reshape(8, 128).T
    ba[:, LN1B_COL:LN1B_COL + 8] = np.asarray(inp["ln1_b"], f).reshape(8, 128).T
    ba[:, LN2W_COL:LN2W_COL + 8] = np.asarray(inp["ln2_w"], f).reshape(8, 128).T
    ba[:, LN2B_COL:LN2B_COL + 8] = np.asarray(inp["ln2_b"], f).reshape(8, 128).T
    W["biasA"] = ba
    ones = np.ones((128, M), np.float32)
    ones[:, 0] = 1.0 / DM            # ones_col used by LN stats -> means
    ones[:, 3] = 0.0                 # zero bias column
    W["onesD"] = ones
    sel = np.zeros((128, 384), np.float32)
    sel[0, 0:64] = 1.0       # pattern A: tile-row 0 -> partitions 0:64
    sel[32, 64:128] = 1.0    #            tile-row 32 -> partitions 64:128
    sel[64, 128 + 0:128 + 64] = 1.0    # pattern B: row 64 -> 0:64
    sel[96, 128 + 64:128 + 128] = 1.0  #            row 96 -> 64:128
    sel[0, 256:384] = 1.0    # ones row for LN broadcasts
    W["selD"] = sel
    return W


def make_in_maps(inputs):
    W = host_pack_weights(inputs)
    x = np.asarray(inputs["x"], np.float32)
    mask = np.asarray(inputs["mask"], np.float32)
    in_maps = []
    for b in range(N_CORES):
        m = dict(W)
        m["xT"] = np.ascontiguousarray(x[b].T)
        m["maskT"] = np.ascontiguousarray(mask[b].reshape(4, 128).T)
        in_maps.append(m)
    return in_maps


_NC = None


def _get_nc():
    global _NC
    if _NC is None:
        _NC = build_program()
    return _NC


def run(inputs, trace=False):
    nc = _get_nc()
    in_maps = make_in_maps(inputs)
    bkr = run_bass_kernel_spmd(nc, in_maps, list(range(N_CORES)), trace=trace)
    out = np.empty((B, M, DM), np.float32)
    for b in range(N_CORES):
        out[b] = bkr.results[b]["outT"].T
    return out, bkr


def kernel(**inputs):
    out, _ = run(inputs)
    return out
